# revision 1
# baseline (speedup 1.0000x reference)
"""Trainium2 Bass kernel for windowed causal multi-head attention.

Problem (hardcoded): x [4, 2048, 1024], 16 heads x 64 dim, rotary embedding,
causal attention with left window 256, fused QKV/out projections.

Sharding: 8 cores = (batch b in 0..3) x (head-group g in 0..1). Each core
computes batch b, heads [g*8, (g+1)*8) and produces a partial output
[2048, 1024] (its head-group's contribution to the out-projection). The host
sums the two partials per batch and adds the output bias.

Device-side layout strategy (transpose-free):
  - Host supplies x^T per batch. Projections compute qT/kT [hd, seq] with
    head-dim on partitions (lhsT = W chunk), and v [seq, hd] naturally
    (lhsT = xT chunk). Rotary rotate_half is a 128x128 constant matmul (R)
    plus two elementwise multiplies against host-precomputed cos/sin rows.
  - Scores are computed transposed, S^T [keys, queries], per head via
    K=64-contraction matmuls (head pairs share the PE array via row tiling).
  - Softmax: P = exp(S^T * scale) (no max subtraction needed: |scores|<~10),
    band masks applied as {0,1} multiplies; denominators via ones-matmuls
    that replicate the per-query sums across partitions; context
    C^T = V^T P^T accumulated over the banded key blocks; normalized with a
    reciprocal multiply.
  - Out projection consumes C^T directly as lhsT (K = head dims).
  - All matmul operands are bf16 (fp32 PSUM accumulate); elementwise and
    output stay fp32 where it matters. Measured rel err ~3.8e-3.
  - Engines execute their queues in order, so emission order is the
    schedule: attention tails are software-pipelined DEPTH iterations
    behind their score matmuls.
"""

import numpy as np

import concourse.bass as bass
import concourse.mybir as mybir
import concourse.tile as tile
from concourse import bacc
from concourse import bass_utils

B, S, E = 4, 2048, 1024
H, D = 16, 64
W = 512          # per-core head-group width (8 heads x 64)
QB = 256         # query block
NQB = S // QB    # 8
NKC = E // 128   # 8 contraction chunks for projections
PAIRS = 4        # head pairs per core (128 cols each)
SCALE = 1.0 / 8.0  # 1/sqrt(D)

F32 = mybir.dt.float32
BF16_MATMULS = True
MDT = mybir.dt.bfloat16 if BF16_MATMULS else mybir.dt.float32r

_STATE = None  # (nc, results cache key)


def _build():
    nc = bacc.Bacc("TRN2", target_bir_lowering=False, debug=False, num_devices=8)

    xtc = nc.dram_tensor("xtc", [NQB, 128, NKC, QB], MDT,
                         kind="ExternalInput").ap()
    wq = nc.dram_tensor("wq", [128, NKC, W], MDT, kind="ExternalInput").ap()
    wk = nc.dram_tensor("wk", [128, NKC, W], MDT, kind="ExternalInput").ap()
    wv = nc.dram_tensor("wv", [128, NKC, W], MDT, kind="ExternalInput").ap()
    wo = nc.dram_tensor("wo", [128, PAIRS, E], MDT, kind="ExternalInput").ap()
    bqc = nc.dram_tensor("bqc", [128, PAIRS], F32, kind="ExternalInput").ap()
    bvr = nc.dram_tensor("bvr", [1, W], MDT, kind="ExternalInput").ap()
    ones1 = nc.dram_tensor("ones1", [1, 128], MDT, kind="ExternalInput").ap()
    onesd = nc.dram_tensor("onesd", [128, 64], MDT, kind="ExternalInput").ap()
    rt = nc.dram_tensor("rt", [128, 128], MDT, kind="ExternalInput").ap()
    cosh = nc.dram_tensor("cosh", [64, S], MDT, kind="ExternalInput").ap()
    sinh = nc.dram_tensor("sinh", [64, S], MDT, kind="ExternalInput").ap()
    masklu = nc.dram_tensor("masklu", [2, 128, 128], MDT, kind="ExternalInput").ap()
    out = nc.dram_tensor("out", [S, E], F32, kind="ExternalOutput").ap()


    with tile.TileContext(nc) as tc:
        with tc.tile_pool(name="res", bufs=1) as res, \
             tc.tile_pool(name="work", bufs=2) as work, \
             tc.tile_pool(name="attn", bufs=2) as attn, \
             tc.tile_pool(name="ps", bufs=4, space="PSUM") as ps, \
             tc.tile_pool(name="ps_s", bufs=2, space="PSUM") as ps_s:

            # --- resident constants / weights ---
            xt_sb = res.tile([128, NKC, S], MDT)     # resident x^T
            cos2_sb = res.tile([128, S], MDT)
            sin2_sb = res.tile([128, S], MDT)
            wq_sb = res.tile([128, NKC, W], MDT)
            wk_sb = res.tile([128, NKC, W], MDT)
            wv_sb = res.tile([128, NKC, W], MDT)
            wo_sb = res.tile([128, PAIRS, E], MDT)
            nc.sync.dma_start(out=wk_sb, in_=wk)
            nc.sync.dma_start(out=xt_sb[:, :, 0:QB], in_=xtc[0])
            nc.sync.dma_start(out=wv_sb, in_=wv)
            nc.sync.dma_start(out=xt_sb[:, :, QB:2 * QB], in_=xtc[1])
            nc.sync.dma_start(out=cos2_sb[0:64, :], in_=cosh)
            nc.sync.dma_start(out=cos2_sb[64:128, :], in_=cosh)
            nc.sync.dma_start(out=sin2_sb[0:64, :], in_=sinh)
            nc.sync.dma_start(out=sin2_sb[64:128, :], in_=sinh)
            nc.sync.dma_start(out=xt_sb[:, :, 2 * QB:3 * QB], in_=xtc[2])
            bqc_sb = res.tile([128, PAIRS], F32)
            bvr_sb = res.tile([1, W], MDT)
            ones1_sb = res.tile([1, 128], MDT)
            onesd_sb = res.tile([128, 64], MDT)
            rt_sb = res.tile([128, 128], MDT)
            masklu_sb = res.tile([128, 2, 128], MDT)
            nc.sync.dma_start(out=bvr_sb, in_=bvr)
            nc.sync.dma_start(out=ones1_sb, in_=ones1)
            nc.sync.dma_start(out=rt_sb, in_=rt)

            kT_sb = res.tile([128, PAIRS, S], MDT)   # rotated K^T per pair
            v_sb = res.tile([128, S // 128, W], MDT)  # V (seq-major tiles)

            # --- Stage A: K^T (rotated) and V for the whole sequence ---
            # Same in-order-queue trick as stage B: the rotation matmul of
            # iteration i is emitted after the projections of iteration i+1.
            a_pend = []

            def emit_ktail():
                kraw, sl_ = a_pend.pop(0)
                ps_rh = ps.tile([128, QB], F32, tag="dc", bufs=2,
                                name=f"ps_rh_a_{sl_.start}")
                nc.tensor.matmul(ps_rh, rt_sb, kraw, start=True, stop=True)
                rhs_sin = work.tile([128, QB], MDT, tag="rhsin", bufs=3,
                                    name=f"rhs_sin_a_{sl_.start}")
                nc.vector.tensor_mul(out=rhs_sin, in0=ps_rh,
                                     in1=sin2_sb[:, sl_])
                nc.vector.tensor_mul(out=kraw, in0=kraw,
                                     in1=cos2_sb[:, sl_])
                nc.vector.tensor_add(
                    out=kT_sb[:, a_pend_c.pop(0), sl_], in0=kraw,
                    in1=rhs_sin)

            a_pend_c = []
            for n8 in range(S // QB):
                sl = slice(n8 * QB, (n8 + 1) * QB)
                xa = xt_sb[:, :, sl]
                if 3 <= n8 + 3 <= 7:
                    pre = n8 + 3
                    nc.sync.dma_start(
                        out=xt_sb[:, :, pre * QB:(pre + 1) * QB],
                        in_=xtc[pre])
                for c in range(PAIRS):
                    ps_k = ps.tile([128, QB], F32, tag="pj", bufs=2,
                                   name=f"ps_k_{n8}_{c}")
                    for kc in range(NKC):
                        nc.tensor.matmul(
                            ps_k,
                            wk_sb[:, kc, c * 128:(c + 1) * 128],
                            xa[:, kc, :],
                            start=(kc == 0), stop=(kc == NKC - 1),
                        )
                    kraw = work.tile([128, QB], MDT, tag="kraw", bufs=3,
                                     name=f"kraw_{n8}_{c}")
                    nc.scalar.activation(
                        out=kraw, in_=ps_k,
                        func=mybir.ActivationFunctionType.Copy)
                    a_pend.append((kraw, sl))
                    a_pend_c.append(c)
                    if len(a_pend) > 1:
                        emit_ktail()
                for sub in range(2):
                    jb = n8 * 2 + sub
                    ps_v = ps.tile([128, W], F32, tag="pj", bufs=2,
                                   name=f"ps_v_{jb}")
                    for kc in range(NKC):
                        nc.tensor.matmul(
                            ps_v,
                            xa[:, kc, sub * 128:(sub + 1) * 128],
                            wv_sb[:, kc, :],
                            start=(kc == 0), stop=False,
                        )
                    nc.tensor.matmul(ps_v, ones1_sb, bvr_sb, start=False,
                                     stop=True)
                    nc.scalar.activation(
                        out=v_sb[:, jb, :], in_=ps_v,
                        func=mybir.ActivationFunctionType.Copy)
            while a_pend:
                emit_ktail()

            # Stage-B-only constants: issue after stage A's DMAs so the
            # first kT matmuls aren't queued behind 4MB of weights.
            nc.sync.dma_start(out=wq_sb, in_=wq)
            nc.sync.dma_start(out=wo_sb, in_=wo)
            nc.sync.dma_start(out=bqc_sb, in_=bqc)
            nc.sync.dma_start(out=onesd_sb, in_=onesd)
            nc.sync.dma_start(
                out=masklu_sb, in_=masklu.rearrange("k p f -> p k f"))

            # --- Stage B: per query block: Q, attention, out-projection ---
            # Engines run their instruction queues IN ORDER, so the emission
            # order is the schedule. Attention tails (den/PV matmuls) for
            # iteration i are emitted after the scores of iteration i+DEPTH
            # so the PE never sits waiting on the ACT exp of iteration i.
            DEPTH = 3
            pending = []          # (qb, c, h2, i0, n_valid, pAB, cn)
            qb_tails_left = {}    # qb -> remaining tail count
            cn_by_qb = {}         # qb -> [cn tiles]

            def emit_wo(qb):
                cts = cn_by_qb.pop(qb)
                for sub in range(2):
                    o_sb = work.tile([128, 1024], F32, tag="o_sb",
                                     name=f"o_sb_{qb}_{sub}")
                    for ncol in range(2):
                        ps_o = ps.tile([128, 512], F32, tag="dc", bufs=2,
                                       name=f"ps_o_{qb}_{sub}_{ncol}")
                        for cc in range(PAIRS):
                            nc.tensor.matmul(
                                ps_o,
                                cts[cc][:, sub * 128:(sub + 1) * 128],
                                wo_sb[:, cc, ncol * 512:(ncol + 1) * 512],
                                start=(cc == 0), stop=(cc == PAIRS - 1))
                        nc.scalar.activation(
                            out=o_sb[:, ncol * 512:(ncol + 1) * 512],
                            in_=ps_o,
                            func=mybir.ActivationFunctionType.Copy)
                    nc.sync.dma_start(
                        out=out[qb * QB + sub * 128:
                                qb * QB + (sub + 1) * 128, :],
                        in_=o_sb)

            def emit_tail():
                qb, c, h2, i0, n_valid, pAB, cn = pending.pop(0)
                qh = slice(h2 * 128, (h2 + 1) * 128)
                ps_dc = ps.tile([128, 2, 128], F32, tag="dc", bufs=2,
                                name=f"ps_dc_{qb}_{c}_{h2}")
                ps_den = ps_dc[:, 0, :]
                ps_c = ps_dc[:, 1, :]
                # Head B lands on partitions 0-63 (col group 0), head A on
                # 64-127 (col group 2, tile_position) so den/C/recip stay
                # partition-aligned with a single recip + multiply. The
                # host swaps Wo's row halves per pair to compensate. The
                # groups stay sequential per bank; adjacent groups overlap
                # on the PE via distinct column groups.
                sides = (((4, slice(0, 64), None, c * 128 + 64)),
                         ((0, slice(64, 128), (0, 64), c * 128)))
                for sb_, hs, tp, v_lo in sides:
                    for k in range(3 - n_valid, 3):
                        st = k == 3 - n_valid
                        sp = k == 2
                        nc.tensor.matmul(
                            ps_den[hs, :], onesd_sb,
                            pAB[:, sb_ + k, :], start=st, stop=sp,
                            tile_position=tp)
                for sb_, hs, tp, v_lo in sides:
                    for k in range(3 - n_valid, 3):
                        jb = i0 - 2 + k
                        st = k == 3 - n_valid
                        sp = k == 2
                        nc.tensor.matmul(
                            ps_c[hs, :],
                            v_sb[:, jb, v_lo:v_lo + 64],
                            pAB[:, sb_ + k, :], start=st, stop=sp,
                            tile_position=tp)
                recip = work.tile([128, 128], F32, tag="recip",
                                  name=f"recip_{qb}_{c}_{h2}")
                nc.vector.reciprocal_approx_fast(out=recip, in_=ps_den)
                nc.vector.tensor_mul(out=cn[:, qh], in0=ps_c, in1=recip)
                qb_tails_left[qb] -= 1
                if qb_tails_left[qb] == 0:
                    emit_wo(qb)

            for qb in range(NQB):
                qsl = slice(qb * QB, (qb + 1) * QB)
                xq = xt_sb[:, :, qsl]
                qb_tails_left[qb] = 2 * PAIRS
                cn_by_qb[qb] = []
                qrots = []
                for c in range(PAIRS):
                    ps_q = ps.tile([128, QB], F32, tag="pj", bufs=2,
                                   name=f"ps_q_{qb}_{c}")
                    for kc in range(NKC):
                        nc.tensor.matmul(
                            ps_q,
                            wq_sb[:, kc, c * 128:(c + 1) * 128],
                            xq[:, kc, :],
                            start=(kc == 0), stop=(kc == NKC - 1),
                        )
                    qraw = work.tile([128, QB], MDT, tag="kraw", bufs=3,
                                     name=f"qraw_{qb}_{c}")
                    nc.scalar.activation(
                        out=qraw, in_=ps_q,
                        func=mybir.ActivationFunctionType.Identity,
                        bias=bqc_sb[:, c:c + 1])
                    ps_rh = ps.tile([128, QB], F32, tag="dc", bufs=2,
                                    name=f"ps_rh_{qb}_{c}")
                    nc.tensor.matmul(ps_rh, rt_sb, qraw, start=True, stop=True)
                    qrot = work.tile([128, QB], MDT, tag="qrot",
                                     name=f"qrot_{qb}_{c}", bufs=3)
                    rhs_sin = work.tile([128, QB], MDT, tag="rhsin", bufs=3,
                                        name=f"rhs_sin_{qb}_{c}")
                    nc.vector.tensor_mul(out=rhs_sin, in0=ps_rh,
                                         in1=sin2_sb[:, qsl])
                    nc.vector.tensor_mul(out=qraw, in0=qraw,
                                         in1=cos2_sb[:, qsl])
                    nc.vector.tensor_add(out=qrot, in0=qraw, in1=rhs_sin)
                    qrots.append(qrot)

                for c in range(PAIRS):
                    cn = work.tile([128, QB], MDT, tag="cn", bufs=8,
                                   name=f"cn_{qb}_{c}")
                    cn_by_qb[qb].append(cn)
                    for h2 in range(2):
                        i0 = qb * 2 + h2
                        qh = slice(h2 * 128, (h2 + 1) * 128)
                        sAB = ps_s.tile([128, 8, 128], F32, tag="sAB",
                                        name=f"sAB_{qb}_{c}_{h2}")
                        pAB = attn.tile([128, 8, 128], MDT, tag="pAB",
                                        bufs=2 + DEPTH,
                                        name=f"pAB_{qb}_{c}_{h2}")
                        n_valid = min(3, i0 + 1)
                        # A (array rows 0-63) and B (rows 64-127) score
                        # matmuls alternate so the PE runs both row groups
                        # concurrently (outputs land in different banks).
                        for k in range(3 - n_valid, 3):
                            jb = i0 - 2 + k
                            for sb_, hs in ((0, slice(0, 64)),
                                            (4, slice(64, 128))):
                                nc.tensor.matmul(
                                    sAB[:, sb_ + k, :],
                                    kT_sb[hs, c, jb * 128:(jb + 1) * 128],
                                    qrots[c][hs, qh], start=True, stop=True)
                        for sb_ in (0, 4):
                            lo = sb_ + 3 - n_valid
                            nc.scalar.activation(
                                out=pAB[:, lo:sb_ + 3, :],
                                in_=sAB[:, lo:sb_ + 3, :],
                                func=mybir.ActivationFunctionType.Exp,
                                scale=SCALE)
                            if n_valid == 3:
                                nc.vector.tensor_mul(
                                    out=pAB[:, sb_:sb_ + 3:2, :],
                                    in0=pAB[:, sb_:sb_ + 3:2, :],
                                    in1=masklu_sb)
                            else:
                                nc.vector.tensor_mul(
                                    out=pAB[:, sb_ + 2, :],
                                    in0=pAB[:, sb_ + 2, :],
                                    in1=masklu_sb[:, 1, :])
                        pending.append((qb, c, h2, i0, n_valid, pAB, cn))
                        if len(pending) > DEPTH:
                            emit_tail()
            while pending:
                emit_tail()

    nc.compile()
    return nc


def _host_consts():
    R64 = np.zeros((64, 64), np.float32)
    for d in range(32):
        R64[d, d + 32] = -1.0
    for d in range(32, 64):
        R64[d, d - 32] = 1.0
    Rblk = np.zeros((128, 128), np.float32)
    Rblk[:64, :64] = R64
    Rblk[64:, 64:] = R64
    rt = np.ascontiguousarray(Rblk.T)

    pv, fv = np.meshgrid(np.arange(128), np.arange(128), indexing="ij")
    masklu = np.stack([(fv <= pv), (fv >= pv)]).astype(np.float32)
    return rt, masklu


def _make_in_maps(x, cos, sin, Wq, bq, Wk, Wv, bv, Wo):
    if BF16_MATMULS:
        import ml_dtypes
        mdt_np = np.dtype(ml_dtypes.bfloat16)
    else:
        mdt_np = np.dtype(np.float32)
    rt, masklu = _host_consts()
    ones1 = np.ones((1, 128), mdt_np)
    onesd = np.ones((128, 64), mdt_np)
    in_maps = []
    for core in range(8):
        b, g = core // 2, core % 2
        gs = slice(g * W, (g + 1) * W)
        cosT = np.ascontiguousarray(cos[b].T)
        sinT = np.ascontiguousarray(sin[b].T)
        xT = x[b].T.astype(mdt_np)  # [1024, 2048]
        xtc = np.ascontiguousarray(
            xT.reshape(8, 128, 8, 256).transpose(2, 1, 0, 3))
        in_maps.append({
            "xtc": xtc,
            "wq": np.ascontiguousarray(
                Wq[:, gs].reshape(8, 128, 512).transpose(1, 0, 2)
            ).astype(mdt_np),
            "wk": np.ascontiguousarray(
                Wk[:, gs].reshape(8, 128, 512).transpose(1, 0, 2)
            ).astype(mdt_np),
            "wv": np.ascontiguousarray(
                Wv[:, gs].reshape(8, 128, 512).transpose(1, 0, 2)
            ).astype(mdt_np),
            "wo": np.ascontiguousarray(
                Wo[gs, :].reshape(4, 2, 64, 1024)[:, ::-1]
                .reshape(4, 128, 1024).transpose(1, 0, 2)
            ).astype(mdt_np),
            "bqc": np.ascontiguousarray(
                bq[gs].reshape(PAIRS, 128).T).astype(np.float32),
            "bvr": bv[gs].reshape(1, W).astype(mdt_np),
            "ones1": ones1,
            "onesd": onesd,
            "rt": rt.astype(mdt_np),
            "cosh": cosT.astype(mdt_np),
            "sinh": sinT.astype(mdt_np),
            "masklu": masklu.astype(mdt_np),
        })
    return in_maps


def _get_nc():
    global _STATE
    if _STATE is None:
        _STATE = _build()
    return _STATE


def run(inputs, trace=False, trace_cores=None):
    """Run the SPMD kernel; returns (full_output, BassKernelResults)."""
    nc = _get_nc()
    in_maps = _make_in_maps(
        inputs["x"], inputs["cos"], inputs["sin"], inputs["Wq"], inputs["bq"],
        inputs["Wk"], inputs["Wv"], inputs["bv"], inputs["Wo"])
    res = bass_utils.run_bass_kernel_spmd(
        nc, in_maps, core_ids=list(range(8)), trace=trace,
        trace_cores=trace_cores)
    mask = np.asarray(inputs["mask"])
    bo = np.asarray(inputs["bo"])
    out = np.zeros((B, S, E), np.float32)
    for core in range(8):
        b = core // 2
        out[b] += res.results[core]["out"]
    out += bo[None, None, :]
    out *= mask[..., None].astype(np.float32)
    return out, res


def kernel(**inputs) -> np.ndarray:
    inputs = {k: np.asarray(v) for k, v in inputs.items()}
    out, _ = run(inputs)
    return out

